# revision 9
# baseline (speedup 1.0000x reference)
"""BitNet transformer block on 8 Trainium2 NeuronCores (Bass/Tile SPMD).

Sharding: data-parallel fold-balanced attention (core i owns query blocks
{i, 15-i} of 16 x 128 tokens; kv-proj token-sharded + AllGather), then
tensor-parallel MLP over INTER/8 with a token-chunked ReduceScatter.
Weights are ternary-quantized on host (exact {-1,0,+1} in bf16) with fp32
per-feature scales applied at PSUM evict. The residual after o_proj is
returned per-core and added during host-side unshard assembly.
"""

import sys

import numpy as np

try:
    import concourse.bass as bass  # noqa: F401
except Exception:  # pragma: no cover
    sys.path.insert(0, "/opt/trn_rl_repo")

import ml_dtypes
import concourse.bass as bass
import concourse.mybir as mybir
import concourse.tile as tile
from concourse import bacc
from concourse.bass_utils import run_bass_kernel_spmd

FP32 = mybir.dt.float32
BF16 = mybir.dt.bfloat16
BF = ml_dtypes.bfloat16

ALPHA = 0.7
EPS = 1e-5
NH = 16          # query heads
NKV = 4          # kv heads
D = 128          # head dim
H = 2048         # hidden
I_TOT = 8192     # mlp intermediate
S = 2048         # sequence
NC = 8           # cores
P = 128
HT = H // P      # 16 hidden tiles
B = S // P       # 16 token blocks
I_LOC = I_TOT // NC   # 1024 intermediate per core
IT = I_LOC // P       # 8 inter tiles per core
TOK = 256             # tokens owned per core (2 blocks)
NCHUNK = 4            # reduce-scatter token chunks
CH = S // NCHUNK      # 512 tokens per chunk

# gathered token order: core i contributes blocks [i, 15-i]
PERM = []
for _i in range(NC):
    PERM += [_i, 15 - _i]

_CACHE = {}


def _build_program():
    nc = bacc.Bacc("TRN2", target_bir_lowering=False, debug=False, num_devices=NC)
    AF = mybir.ActivationFunctionType
    ALU = mybir.AluOpType
    rg = [list(range(NC))]

    # ---------------- inputs ----------------
    def dram_in(name, shape, dt=FP32):
        return nc.dram_tensor(name, shape, dt, kind="ExternalInput")

    xT_own = dram_in("xT_own", [P, HT, TOK])          # fp32, x^T own token cols
    xT_kv = dram_in("xT_kv", [P, HT, TOK])            # fp32, x^T kv token cols
    cos_q = dram_in("cos_q", [P, TOK])
    sin_q = dram_in("sin_q", [P, TOK])
    cos_k = dram_in("cos_k", [P, TOK])
    sin_k = dram_in("sin_k", [P, TOK])
    wq_in = dram_in("wq", [HT, P, HT, P], BF16)       # [f, p, kt, c]
    wk_in = dram_in("wk", [NKV, P, HT, P], BF16)
    wv_in = dram_in("wv", [P, HT, NKV * D], BF16)     # rhs layout
    wo_in = dram_in("wo", [HT, P, HT, P], BF16)
    wg_in = dram_in("wg", [IT, P, HT, P], BF16)
    wu_in = dram_in("wu", [IT, P, HT, P], BF16)
    wd_in = dram_in("wd", [P, IT, H], BF16)           # [p, it, ho]
    aq_in = dram_in("aq", [P, HT])                    # scales (feature-tiled)
    ak_in = dram_in("ak", [P, NKV])
    av_in = dram_in("av", [P, NKV])
    ao_in = dram_in("ao", [P, HT])
    ag_in = dram_in("ag", [P, IT])
    au_in = dram_in("au", [P, IT])
    ad_in = dram_in("ad", [P, HT])
    rT_in = dram_in("rT", [P, P], BF16)               # rope rotate-half perm^T
    mask_in = dram_in("maskt", [P, B, TOK], BF16)     # per-core causal masks
    ones_f_in = dram_in("ones_f", [P, P])             # fp32 ones
    ones_b_in = dram_in("ones_b", [P, 1], BF16)       # bf16 ones column

    outT = nc.dram_tensor("outT", [TOK, H], FP32, kind="ExternalOutput")
    xmidT = nc.dram_tensor("xmidT", [P, HT, TOK], FP32, kind="ExternalOutput")

    with tile.TileContext(nc) as tc:
        # ---- persistent-ish SBUF (explicit pools per lifetime) ----
        const = tc.alloc_tile_pool(name="const", bufs=1)
        ones_f = const.tile([P, P], FP32)
        ones_b = const.tile([P, 1], BF16)
        rT = const.tile([P, P], BF16)
        cq = const.tile([P, TOK], FP32)
        sq_ = const.tile([P, TOK], FP32)
        ck = const.tile([P, TOK], FP32)
        sk = const.tile([P, TOK], FP32)
        aq = const.tile([P, HT], FP32)
        ak = const.tile([P, NKV], FP32)
        av = const.tile([P, NKV], FP32)
        ao = const.tile([P, HT], FP32)
        ag = const.tile([P, IT], FP32)
        au = const.tile([P, IT], FP32)
        ad = const.tile([P, HT], FP32)
        eps_t = const.tile([P, 1], FP32)
        nc.any.memset(eps_t[:], EPS)
        for dst, src in [(ones_f, ones_f_in), (ones_b, ones_b_in), (rT, rT_in),
                         (cq, cos_q), (sq_, sin_q), (ck, cos_k), (sk, sin_k),
                         (aq, aq_in), (ak, ak_in), (av, av_in), (ao, ao_in),
                         (ag, ag_in), (au, au_in), (ad, ad_in)]:
            nc.sync.dma_start(dst[:], src[:])

        midpool = tc.alloc_tile_pool(name="midpool", bufs=1)
        x_mid = midpool.tile([P, HT, TOK], FP32)
        h2 = midpool.tile([P, HT, TOK], BF16)
        opool = tc.alloc_tile_pool(name="opool", bufs=1)
        o_all = opool.tile([P, NH, TOK], BF16)
        xopool = tc.alloc_tile_pool(name="xopool", bufs=1)
        xo = xopool.tile([P, HT, TOK], FP32)     # x^T own
        nc.sync.dma_start(xo[:], xT_own[:])
        qkvpool = tc.alloc_tile_pool(name="qkvpool", bufs=1)
        q_rope = qkvpool.tile([P, NH, TOK], BF16)
        k_all = qkvpool.tile([P, NKV, B, P], BF16)   # [d, kvh, blk, tok]
        v_all = qkvpool.tile([P, NKV, B, P], BF16)   # [tok, kvh, blk, d]

        def rmsnorm(src3d, out3d, psp, tmp):
            """src3d [P,HT,TOK] fp32 -> out3d [P,HT,TOK] bf16 (x * rsqrt(mean x^2 + eps))"""
            ssq = psp.tile([1, TOK], FP32, name="ssq")
            for kt in range(HT):
                sqv = tmp.tile([P, TOK], FP32, name="sqv")
                nc.vector.tensor_mul(sqv[:], src3d[:, kt, :], src3d[:, kt, :])
                nc.tensor.matmul(ssq[:], ones_f[:, 0:1], sqv[:],
                                 start=(kt == 0), stop=(kt == HT - 1))
            ms = tmp.tile([1, TOK], FP32, name="ms")
            nc.scalar.activation(ms[:], ssq[:], AF.Identity, bias=eps_t[0:1, :], scale=1.0 / H)
            rec = tmp.tile([1, TOK], FP32, name="rec")
            nc.vector.reciprocal(rec[:], ms[:])
            rsq = tmp.tile([1, TOK], FP32, name="rsq")
            nc.scalar.activation(rsq[:], rec[:], AF.Sqrt)
            bc = psp.tile([P, TOK], FP32, name="bc")
            nc.tensor.matmul(bc[:], ones_f[0:1, :], rsq[:], start=True, stop=True)
            for kt in range(HT):
                nc.vector.tensor_mul(out3d[:, kt, :], src3d[:, kt, :], bc[:])

        # ================= phase 1: ln1 =================
        with tc.tile_pool(name="p1", bufs=2) as p1sb, \
             tc.tile_pool(name="p1ps", bufs=1, space="PSUM") as p1ps, \
             tc.tile_pool(name="hpool", bufs=1) as hpool:
            h_own = hpool.tile([P, HT, TOK], BF16)
            h_kv = hpool.tile([P, HT, TOK], BF16)
            xkv = hpool.tile([P, HT, TOK], FP32)
            nc.sync.dma_start(xkv[:], xT_kv[:])
            rmsnorm(xo, h_own, p1ps, p1sb)
            rmsnorm(xkv, h_kv, p1ps, p1sb)

            # ============= phase 2: q/k/v proj + rope =============
            with tc.tile_pool(name="wq_pool", bufs=3) as wp, \
                 tc.tile_pool(name="p2ps", bufs=2, space="PSUM") as p2ps, \
                 tc.tile_pool(name="p2sb", bufs=3) as p2sb, \
                 tc.tile_pool(name="wvres", bufs=1) as wvres:
                # q projection: per head f -> psum [P, TOK]
                for f in range(NH):
                    wt = wp.tile([P, HT, P], BF16, name="wt")
                    nc.sync.dma_start(wt[:], wq_in[f])
                    ps = p2ps.tile([P, TOK], FP32, name="qps")
                    for kt in range(HT):
                        nc.tensor.matmul(ps[:], wt[:, kt, :], h_own[:, kt, :],
                                         start=(kt == 0), stop=(kt == HT - 1))
                    qs = p2sb.tile([P, TOK], BF16, name="qs")
                    nc.scalar.activation(qs[:], ps[:], AF.Copy, scale=aq[:, f:f + 1])
                    # rope: q*cos + (R q)*sin
                    rot = p2ps.tile([P, TOK], FP32, name="rot")
                    nc.tensor.matmul(rot[:], rT[:], qs[:], start=True, stop=True)
                    t1 = p2sb.tile([P, TOK], FP32, name="t1")
                    nc.vector.tensor_mul(t1[:], rot[:], sq_[:])
                    t2 = p2sb.tile([P, TOK], FP32, name="t2")
                    nc.vector.tensor_mul(t2[:], qs[:], cq[:])
                    nc.vector.tensor_add(q_rope[:, f, :], t1[:], t2[:])
                # k projection (kv tokens)
                for f in range(NKV):
                    wt = wp.tile([P, HT, P], BF16, name="wt")
                    nc.sync.dma_start(wt[:], wk_in[f])
                    ps = p2ps.tile([P, TOK], FP32, name="qps")
                    for kt in range(HT):
                        nc.tensor.matmul(ps[:], wt[:, kt, :], h_kv[:, kt, :],
                                         start=(kt == 0), stop=(kt == HT - 1))
                    ks = p2sb.tile([P, TOK], BF16, name="qs")
                    nc.scalar.activation(ks[:], ps[:], AF.Copy, scale=ak[:, f:f + 1])
                    rot = p2ps.tile([P, TOK], FP32, name="rot")
                    nc.tensor.matmul(rot[:], rT[:], ks[:], start=True, stop=True)
                    t1 = p2sb.tile([P, TOK], FP32, name="t1")
                    nc.vector.tensor_mul(t1[:], rot[:], sk[:])
                    t2 = p2sb.tile([P, TOK], FP32, name="t2")
                    nc.vector.tensor_mul(t2[:], ks[:], ck[:])
                    # k_rope local: write into k bounce staging tile
                    kst = p2sb.tile([P, TOK], BF16, name=f"kst{f}", tag="kst")
                    nc.vector.tensor_add(kst[:], t1[:], t2[:])
                    nc.sync.dma_start(nc_k_in_view(nc, f), kst[:])
                # v projection: swapped operands -> natural [tok, d] layout
                wv_sb = wvres.tile([P, HT, NKV * D], BF16)
                nc.sync.dma_start(wv_sb[:], wv_in[:])
                for tb in range(2):
                    ps = p2ps.tile([P, NKV * D], FP32, name="vps")
                    for kt in range(HT):
                        nc.tensor.matmul(ps[:], h_kv[:, kt, tb * P:(tb + 1) * P],
                                         wv_sb[:, kt, :],
                                         start=(kt == 0), stop=(kt == HT - 1))
                    vs = p2sb.tile([P, NKV, D], BF16, name="vs")
                    nc.scalar.activation(vs[:], ps[:].rearrange("p (h d) -> p h d", h=NKV),
                                         AF.Copy)
                    nc.sync.dma_start(nc_v_in_view(nc, tb), vs[:])

        # ============= phase 3: allgather k, v =============
        k_gath = nc.dram_tensor("k_gath", [NC * P, NKV, TOK], BF16, addr_space="Shared")
        v_gath = nc.dram_tensor("v_gath", [NC * P, 2, NKV, D], BF16, addr_space="Shared")
        nc.gpsimd.collective_compute(
            "AllGather", mybir.AluOpType.bypass, ins=[nc.k_in_t[:]],
            outs=[k_gath[:]], replica_groups=rg)
        nc.gpsimd.collective_compute(
            "AllGather", mybir.AluOpType.bypass, ins=[nc.v_in_t[:]],
            outs=[v_gath[:]], replica_groups=rg)
        kg = k_gath[:].rearrange("(r p) h t -> r p h t", r=NC)
        vg = v_gath[:].rearrange("(r p) tb h d -> r p tb h d", r=NC)
        for r in range(NC):
            nc.sync.dma_start(
                k_all[:, :, 2 * r:2 * r + 2, :],
                kg[r].rearrange("p h (b t) -> p h b t", b=2))
            for tb in range(2):
                nc.sync.dma_start(v_all[:, :, 2 * r + tb, :], vg[r][:, tb, :, :])

        # ============= phase 4: attention =============
        with tc.tile_pool(name="mskp", bufs=1) as mskp, \
             tc.tile_pool(name="a_ps", bufs=3, space="PSUM") as a_ps, \
             tc.tile_pool(name="o_ps", bufs=2, space="PSUM") as o_ps, \
             tc.tile_pool(name="l_ps", bufs=2, space="PSUM") as l_ps, \
             tc.tile_pool(name="bc_ps", bufs=1, space="PSUM") as bc_ps, \
             tc.tile_pool(name="a_sb", bufs=3) as a_sb:
            msk = mskp.tile([P, B, TOK], BF16)
            nc.sync.dma_start(msk[:], mask_in[:])
            for hh in range(NH):
                kvh = hh // (NH // NKV)
                ops = o_ps.tile([P, TOK], FP32, name="ops")
                lps = l_ps.tile([1, TOK], FP32, name="lps")
                for kb in range(B):
                    sps = a_ps.tile([P, TOK], FP32, name="sps")
                    nc.tensor.matmul(sps[:], k_all[:, kvh, kb, :], q_rope[:, hh, :],
                                     start=True, stop=True)
                    pm = a_sb.tile([P, TOK], BF16, name="pm")
                    nc.scalar.activation(pm[:], sps[:], AF.Exp)
                    pmm = a_sb.tile([P, TOK], BF16, name="pmm")
                    nc.vector.tensor_mul(pmm[:], pm[:], msk[:, kb, :])
                    nc.tensor.matmul(lps[:], ones_b[:], pmm[:],
                                     start=(kb == 0), stop=(kb == B - 1))
                    nc.tensor.matmul(ops[:], v_all[:, kvh, kb, :], pmm[:],
                                     start=(kb == 0), stop=(kb == B - 1))
                lsb = a_sb.tile([1, TOK], FP32, name="lsb")
                nc.scalar.activation(lsb[:], lps[:], AF.Copy)
                linv = a_sb.tile([1, TOK], FP32, name="linv")
                nc.vector.reciprocal(linv[:], lsb[:])
                bca = bc_ps.tile([P, TOK], FP32, name="bca")
                nc.tensor.matmul(bca[:], ones_f[0:1, :], linv[:], start=True, stop=True)
                osb = a_sb.tile([P, TOK], FP32, name="osb")
                nc.scalar.activation(osb[:], ops[:], AF.Copy, scale=av[:, kvh:kvh + 1])
                nc.vector.tensor_mul(o_all[:, hh, :], osb[:], bca[:])
        qkvpool.release()

        # ============= phase 5: o_proj + residual + ln2 =============
        with tc.tile_pool(name="wo_pool", bufs=3) as wop, \
             tc.tile_pool(name="p5ps", bufs=2, space="PSUM") as p5ps, \
             tc.tile_pool(name="p5sb", bufs=3) as p5sb:
            for f in range(HT):
                wt = wop.tile([P, HT, P], BF16, name="wt")
                nc.sync.dma_start(wt[:], wo_in[f])
                ps = p5ps.tile([P, TOK], FP32, name="ops5")
                for kt in range(HT):
                    nc.tensor.matmul(ps[:], wt[:, kt, :], o_all[:, kt, :],
                                     start=(kt == 0), stop=(kt == HT - 1))
                xs = p5sb.tile([P, TOK], FP32, name="xs")
                nc.scalar.activation(xs[:], ps[:], AF.Copy, scale=ao[:, f:f + 1])
                nc.vector.tensor_add(x_mid[:, f, :], xs[:], xo[:, f, :])
            nc.sync.dma_start(xmidT[:], x_mid[:])
            rmsnorm(x_mid, h2, p5ps, p5sb)
        xopool.release()
        opool.release()

        # ============= phase 6: allgather h2 =============
        h2_in = nc.dram_tensor("h2_in", [P, HT, TOK], BF16)
        h2_gath = nc.dram_tensor("h2_gath", [NC * P, HT, TOK], BF16,
                                 addr_space="Shared")
        nc.sync.dma_start(h2_in[:], h2[:])
        nc.gpsimd.collective_compute(
            "AllGather", mybir.AluOpType.bypass, ins=[h2_in[:]],
            outs=[h2_gath[:]], replica_groups=rg)
        midpool.release()
        h2g = h2_gath[:].rearrange("(r p) kt t -> r p kt t", r=NC)

        # ============= phase 7: MLP (TP over inter) + RS =============
        with tc.tile_pool(name="wd_res", bufs=1) as wdres, \
             tc.tile_pool(name="h2c_pool", bufs=2) as h2cp, \
             tc.tile_pool(name="m_pool", bufs=2) as mp, \
             tc.tile_pool(name="wgu_pool", bufs=3) as wgup, \
             tc.tile_pool(name="p7ps", bufs=2, space="PSUM") as p7ps, \
             tc.tile_pool(name="p7sb", bufs=3) as p7sb:
            wd_sb = wdres.tile([P, IT, H], BF16)
            nc.sync.dma_start(wd_sb[:], wd_in[:])
            rs_outs = []
            for c in range(NCHUNK):
                h2c = h2cp.tile([P, HT, CH], BF16, name="h2c")
                for j in range(2):
                    nc.sync.dma_start(h2c[:, :, j * TOK:(j + 1) * TOK],
                                      h2g[2 * c + j])
                m_all = mp.tile([P, IT, CH], BF16, name="m_all")
                for f in range(IT):
                    wtg = wgup.tile([P, HT, P], BF16, name="wtg")
                    nc.sync.dma_start(wtg[:], wg_in[f])
                    gps = p7ps.tile([P, CH], FP32, name="gps")
                    for kt in range(HT):
                        nc.tensor.matmul(gps[:], wtg[:, kt, :], h2c[:, kt, :],
                                         start=(kt == 0), stop=(kt == HT - 1))
                    wtu = wgup.tile([P, HT, P], BF16, name="wtu")
                    nc.sync.dma_start(wtu[:], wu_in[f])
                    ups = p7ps.tile([P, CH], FP32, name="ups")
                    for kt in range(HT):
                        nc.tensor.matmul(ups[:], wtu[:, kt, :], h2c[:, kt, :],
                                         start=(kt == 0), stop=(kt == HT - 1))
                    gr = p7sb.tile([P, CH], FP32, name="gr")
                    nc.scalar.activation(gr[:], gps[:], AF.Relu, scale=ag[:, f:f + 1])
                    gc = p7sb.tile([P, CH], FP32, name="gc")
                    nc.scalar.activation(gc[:], gps[:], AF.Copy, scale=ag[:, f:f + 1])
                    us = p7sb.tile([P, CH], FP32, name="us")
                    nc.scalar.activation(us[:], ups[:], AF.Copy, scale=au[:, f:f + 1])
                    g2 = p7sb.tile([P, CH], FP32, name="g2")
                    nc.vector.tensor_mul(g2[:], gr[:], gc[:])
                    nc.vector.tensor_mul(m_all[:, f, :], g2[:], us[:])
                rs_in = nc.dram_tensor(f"rs_in_{c}", [H, CH], FP32)
                rs_iv = rs_in[:].rearrange("(f p) t -> f p t", p=P)
                for f in range(HT):
                    dps = p7ps.tile([P, CH], FP32, name="dps")
                    for it in range(IT):
                        nc.tensor.matmul(dps[:], wd_sb[:, it, f * P:(f + 1) * P],
                                         m_all[:, it, :],
                                         start=(it == 0), stop=(it == IT - 1))
                    dn = p7sb.tile([P, CH], FP32, name="dn")
                    nc.scalar.activation(dn[:], dps[:], AF.Copy, scale=ad[:, f:f + 1])
                    nc.sync.dma_start(rs_iv[f], dn[:])
                rs_out = nc.dram_tensor(f"rs_out_{c}", [TOK, CH], FP32)
                nc.gpsimd.collective_compute(
                    "ReduceScatter", mybir.AluOpType.add, ins=[rs_in[:]],
                    outs=[rs_out[:]], replica_groups=rg)
                rs_outs.append(rs_out)
            for c, rs_out in enumerate(rs_outs):
                nc.sync.dma_start(outT[:, c * CH:(c + 1) * CH], rs_out[:])

        const.release()

    nc.finalize()
    return nc


def nc_k_in_view(nc, f):
    if not hasattr(nc, "k_in_t"):
        nc.k_in_t = nc.dram_tensor("k_in", [P, NKV, TOK], BF16)
    return nc.k_in_t[:, f, :]


def nc_v_in_view(nc, tb):
    if not hasattr(nc, "v_in_t"):
        nc.v_in_t = nc.dram_tensor("v_in", [P, 2, NKV, D], BF16)
    return nc.v_in_t[:, tb, :, :]


def _ternary(w, fold_row=None):
    """Quantize [O, Hin] fp32 -> (ternary fp32 {-1,0,1}, absmean [O])."""
    w = np.asarray(w, dtype=np.float32)
    am = np.mean(np.abs(w), axis=1)
    t = np.sign(w) * (np.abs(w) > ALPHA * am[:, None]).astype(np.float32)
    if fold_row is not None:
        t = t * fold_row[None, :]
    return t, am


def _wlhsT(tern, n_f):
    """ternary [O, Hin] -> lhsT input layout [f, p, kt, c] bf16 (tile (kt,f):
    rows Hin-chunk kt, cols O-chunk f)."""
    o, hin = tern.shape
    kt = hin // P
    assert n_f * P == o
    wT = np.ascontiguousarray(tern.T)  # [Hin, O]
    return np.ascontiguousarray(
        wT.reshape(kt, P, n_f, P).transpose(2, 1, 0, 3)).astype(BF)


def _scale_tiles(a):
    """[O] -> [P, O//P] with column f = features f*128..f*128+127."""
    return np.ascontiguousarray(a.reshape(-1, P).T).astype(np.float32)


def _pcol(x2d):
    """[K, T] -> [P, K//P, T] (partition-major for direct DMA)."""
    k, t = x2d.shape
    return np.ascontiguousarray(
        x2d.reshape(k // P, P, t).transpose(1, 0, 2)).astype(np.float32)


def kernel(x, cos, sin, wq, wk, wv, wo, wg, wu, wd, ln1_w, ln2_w):
    x = np.asarray(x, dtype=np.float32)
    b, s, hdim = x.shape
    assert (b, s, hdim) == (1, S, H)

    if "nc" not in _CACHE:
        _CACHE["nc"] = _build_program()
    nc = _CACHE["nc"]

    ln1 = np.asarray(ln1_w, dtype=np.float32)
    ln2 = np.asarray(ln2_w, dtype=np.float32)

    tq, amq = _ternary(wq, fold_row=ln1)
    tk, amk = _ternary(wk, fold_row=ln1)
    tv, amv = _ternary(wv, fold_row=ln1)
    to, amo = _ternary(wo)
    tg, amg = _ternary(wg, fold_row=ln2)
    tu, amu = _ternary(wu, fold_row=ln2)
    td, amd = _ternary(wd)

    wq_h = _wlhsT(tq, NH)
    wk_h = _wlhsT(tk, NKV)
    wo_h = _wlhsT(to, HT)
    # v as rhs: [p, kt, O] from tv.T [H, 512]
    wv_h = np.ascontiguousarray(
        tv.T.reshape(HT, P, NKV * D).transpose(1, 0, 2)).astype(BF)
    wg_h = _wlhsT(tg, I_TOT // P)   # [64, P, HT, P]; slice per core below
    wu_h = _wlhsT(tu, I_TOT // P)
    # wd lhsT tiles (it, f): rows inter, cols ho: [p, it_global, H]
    wd_h = np.ascontiguousarray(
        td.T.reshape(I_TOT // P, P, H).transpose(1, 0, 2)).astype(BF)  # [P,64,H]

    aq_h = _scale_tiles(amq / np.sqrt(np.float32(D)))
    ak_h = _scale_tiles(amk)
    av_h = _scale_tiles(amv)
    ao_h = _scale_tiles(amo)
    ag_h = _scale_tiles(amg)   # [P, 64]
    au_h = _scale_tiles(amu)
    ad_h = _scale_tiles(amd)

    x2 = x[0]                      # [S, H]
    xT = np.ascontiguousarray(x2.T)  # [H, S]
    cosT = np.ascontiguousarray(np.asarray(cos, np.float32)[0, 0].T)  # [D, S]
    sinT = np.ascontiguousarray(np.asarray(sin, np.float32)[0, 0].T)

    R = np.zeros((P, P), np.float32)
    for m in range(64):
        R[m, m + 64] = -1.0
        R[m + 64, m] = 1.0
    rT_h = np.ascontiguousarray(R.T).astype(BF)
    ones_f = np.ones((P, P), np.float32)
    ones_b = np.ones((P, 1), np.float32).astype(BF)
    triu = np.triu(np.ones((P, P), np.float32))  # [k, q] keep k<=q

    in_maps = []
    for i in range(NC):
        blo, bhi = i, 15 - i
        own_cols = np.r_[blo * P:(blo + 1) * P, bhi * P:(bhi + 1) * P]
        kv_cols = np.arange(2 * i * P, (2 * i + 2) * P)
        # causal mask table [P(k), B(kb), TOK(q: lo|hi)]
        msk = np.zeros((B, P, TOK), np.float32)
        for kb in range(B):
            for j, qb in enumerate((blo, bhi)):
                sl = slice(j * P, (j + 1) * P)
                if kb < qb:
                    msk[kb, :, sl] = 1.0
                elif kb == qb:
                    msk[kb, :, sl] = triu
        msk_h = np.ascontiguousarray(msk.transpose(1, 0, 2)).astype(BF)

        islice = slice(i * IT, (i + 1) * IT)
        in_maps.append({
            "xT_own": _pcol(xT[:, own_cols]),
            "xT_kv": _pcol(xT[:, kv_cols]),
            "cos_q": np.ascontiguousarray(cosT[:, own_cols]),
            "sin_q": np.ascontiguousarray(sinT[:, own_cols]),
            "cos_k": np.ascontiguousarray(cosT[:, kv_cols]),
            "sin_k": np.ascontiguousarray(sinT[:, kv_cols]),
            "wq": wq_h, "wk": wk_h, "wv": wv_h, "wo": wo_h,
            "wg": np.ascontiguousarray(wg_h[islice]),
            "wu": np.ascontiguousarray(wu_h[islice]),
            "wd": np.ascontiguousarray(wd_h[:, islice, :]),
            "aq": aq_h, "ak": ak_h, "av": av_h, "ao": ao_h,
            "ag": np.ascontiguousarray(ag_h[:, islice]),
            "au": np.ascontiguousarray(au_h[:, islice]),
            "ad": ad_h,
            "rT": rT_h, "maskt": msk_h,
            "ones_f": ones_f, "ones_b": ones_b,
        })

    res = run_bass_kernel_spmd(nc, in_maps, list(range(NC)))
    _CACHE["last_result"] = res

    # -------- host assembly --------
    down_T = np.concatenate([res.results[i]["outT"] for i in range(NC)], axis=0)
    xmid_T = np.concatenate(
        [res.results[i]["xmidT"].transpose(1, 0, 2).reshape(H, TOK)
         for i in range(NC)], axis=1)
    tot = down_T + xmid_T          # [H(out), S gathered]
    out_T = np.empty_like(tot)
    for j, blk in enumerate(PERM):
        out_T[:, blk * P:(blk + 1) * P] = tot[:, j * P:(j + 1) * P]
    return np.ascontiguousarray(out_T.T).reshape(1, S, H).astype(np.float32)


if __name__ == "__main__":
    nc = _build_program()
    print("build OK; instructions:",
          sum(len(b.instructions) for f in nc.m.functions for b in f.blocks))
